# revision 1
# baseline (speedup 1.0000x reference)
"""Trainium2 Bass kernel for nn_AutoCorrelation (AutoCorrelation attention, training path).

Algorithm (per core; data-parallel over batch B=8 across 8 cores):
  1. Q, K viewed as (L=1536, D=H*E=1024). Packed real DFT along L via PE matmuls
     against a baked cos/sin basis W (L x L packed: cols 0..768 = cos f, cols
     769..1535 = sin f=1..767).
  2. Cross-spectrum S[f] = sum_ch QF * conj(KF) via fused DVE
     tensor_tensor_reduce reading the DFT results directly from PSUM.
  3. mean_value = irfft(S) (packed form) via DVE multiply-reduce against the
     same resident W.
  4. AllReduce(sum) of mean_value (6KB) across the 8 cores -> shared top-7
     delay indices via the DVE max/max_index (top-8) ops.
  5. Per-core softmax weights from own mean_value at the shared indices.
  6. Roll-aggregate out[t] = sum_i w_i * v[(t+s_i) % L] as a block-circulant
     matmul: 12 distinct 128x128 weight blocks built on-device with
     iota-compares against the (runtime) shifts; out_T = sum_U Wblk[(U-T)%12]^T @ V_U.

No dynamic addressing anywhere; the data-dependent values only enter via
compare-against-scalar ops and the matmul weight blocks.
"""

import numpy as np

import concourse.bass as bass
import concourse.mybir as mybir
import concourse.tile as tile
from concourse import bacc
from concourse import bass_utils

B, L, H, E = 8, 1536, 16, 64
D = H * E            # 1024
P = 128
NC = L // P          # 12 chunks
NF = L // 2 + 1      # 769 rfft bins
TOPK = 7
F32 = mybir.dt.float32

# matmul compute dtype tag: float32r = full-rate fp32 (reduced internal
# precision), float32 = exact but 4 cycles/row.
MM_DTYPE = mybir.dt.float32r

AL = mybir.AluOpType


def _build_w_sbuf_layout() -> np.ndarray:
    """W[l, m] packed DFT basis, laid out host-side exactly as the SBUF tile:
    out[p, (mi*NC + li)*P + j] = W[li*P + p, mi*P + j],  shape (P, NC*NC*P)."""
    l = np.arange(L, dtype=np.float64)[:, None]
    f_cos = np.arange(NF, dtype=np.float64)[None, :]
    f_sin = np.arange(1, L - NF + 1, dtype=np.float64)[None, :]
    Wc = np.cos(2.0 * np.pi * l * f_cos / L)
    Ws = np.sin(2.0 * np.pi * l * f_sin / L)
    W = np.concatenate([Wc, Ws], axis=1).astype(np.float32)  # (L, L)
    # chunk to SBUF layout
    out = np.empty((P, NC * NC * P), np.float32)
    for mi in range(NC):
        for li in range(NC):
            out[:, (mi * NC + li) * P:(mi * NC + li + 1) * P] = (
                W[li * P:(li + 1) * P, mi * P:(mi + 1) * P]
            )
    return out


def _mm(ap):
    return ap.bitcast(MM_DTYPE) if MM_DTYPE != F32 else ap


def build_program(single_core: bool = False) -> bass.Bass:
    # single_core=True replaces the AllReduce with a DRAM copy (for TimelineSim)
    nc = bacc.Bacc(
        "TRN2",
        target_bir_lowering=False,
        debug=False,
        num_devices=1 if single_core else B,
        name="autocorr",
        dynamic_dma_scratch_size=512,
    )

    q_in = nc.dram_tensor("q", [L, D], F32, kind="ExternalInput")
    k_in = nc.dram_tensor("k", [L, D], F32, kind="ExternalInput")
    v_in = nc.dram_tensor("v", [L, D], F32, kind="ExternalInput")
    out_dram = nc.dram_tensor("out", [L, D], F32, kind="ExternalOutput")
    w_dram = nc.inline_tensor(_build_w_sbuf_layout(), name="wdft")

    alpha = 1.0 / (L * D)

    with tile.TileContext(nc) as tc:
        with (
            tc.tile_pool(name="misc", bufs=1) as misc,
            tc.tile_pool(name="dram", bufs=1, space="DRAM") as dram,
            tc.tile_pool(name="outp", bufs=3) as outp,
        ):
            # ---- tiles that live across phases ----
            s2d = misc.tile([P, NC], F32, tag="s2d")       # packed spectrum S'
            mv2d = misc.tile([P, NC], F32, tag="mv2d")     # own mean_value
            junk = misc.tile([P, L], F32, tag="junk")      # product scratch
            s768 = misc.tile([1, 1], F32, tag="s768")
            sbc = misc.tile([P, L], F32, tag="sbc")        # S' row-broadcast
            jk2 = misc.tile([P, D], F32, tag="jk2")        # ACT reduce dump
            bm = misc.tile([1, L], F32, tag="bm")          # batch-summed mv
            # DRAM bounces stored TRANSPOSED (c-major) so reads are contiguous
            ds1 = dram.tile([NC, P], F32)                  # S' flatten bounce
            cc_in = dram.tile([NC, P], F32)
            cc_out = dram.tile([NC, P], F32)

            with (
                tc.tile_pool(name="wpool", bufs=1) as wpool,
                tc.tile_pool(name="qkpool", bufs=1) as qkpool,
                tc.tile_pool(name="dftpsum", bufs=2, space="PSUM") as dftpsum,
            ):
                wbig = wpool.tile([P, NC * NC * P], F32, tag="wbig")
                qbig = qkpool.tile([P, NC * D], F32, tag="qbig")
                kbig = qkpool.tile([P, NC * D], F32, tag="kbig")

                # ---- loads: W[m=0] first (gates the first matmuls), then
                # Q/K (gate every pair), then remaining W in consumption order
                def w_load(mi):
                    nc.sync.dma_start(
                        _mm(wbig[:, mi * NC * P:(mi + 1) * NC * P]),
                        _mm(w_dram[:, mi * NC * P:(mi + 1) * NC * P]),
                    )
                w_load(0)
                for li in range(NC):
                    nc.sync.dma_start(
                        _mm(qbig[:, li * D:(li + 1) * D]),
                        _mm(q_in[li * P:(li + 1) * P, :]),
                    )
                    nc.sync.dma_start(
                        _mm(kbig[:, li * D:(li + 1) * D]),
                        _mm(k_in[li * P:(li + 1) * P, :]),
                    )
                for mi in [6, 1, 7, 2, 8, 3, 9, 4, 10, 5, 11]:
                    w_load(mi)

                # ---- DFT + cross-spectrum, m-chunks in Re/Im pair order ----
                qf_t: dict[int, object] = {}
                kf_t: dict[int, object] = {}
                tre = misc.tile([P, 1], F32, tag="tre")
                tim = misc.tile([P, 1], F32, tag="tim")

                # DVE reads at most one PSUM operand, so stage each KF tile
                # into SBUF (ACT engine) before the DVE multiply-reduces.
                # sbc is dead until the irfft, reuse its first D columns.
                kstage = sbc[:, 0:D]

                t2 = misc.tile([P, 1], F32, tag="t2")
                t3 = misc.tile([P, 1], F32, tag="t3")
                AX = mybir.AxisListType.X

                def mul_red(dst, a_ap, b_ap, rows=None):
                    """dst[:,0:1] = sum over free of a*b.

                    DVE does the elementwise product; the (otherwise idle)
                    ACT engine does the reduction via activation accum_out."""
                    jd = junk[:, 0:D] if rows is None else junk[0:rows, 0:D]
                    j2 = jk2[:, 0:D] if rows is None else jk2[0:rows, 0:D]
                    nc.vector.tensor_tensor(jd, a_ap, b_ap, AL.mult)
                    nc.scalar.activation(
                        out=j2, in_=jd,
                        func=mybir.ActivationFunctionType.Copy,
                        accum_out=dst,
                    )

                def emit_pair_products(r):
                    qre, qim = qf_t[r], qf_t[r + 6]
                    kre, kim = kf_t[r], kf_t[r + 6]
                    nc.scalar.copy(kstage, kre[:])
                    mul_red(tre[:, 0:1], qre[:], kstage)   # sum QRe.KRe
                    mul_red(tim[:, 0:1], qim[:], kstage)   # sum QIm.KRe
                    nc.scalar.copy(kstage, kim[:])
                    if r == 0:
                        # s768 = alpha * sum_ch QRe[768]*KRe[768] (tile 6, row 0)
                        mul_red(s768[0:1, 0:1], qim[0:1, :], kstage[0:1, :], rows=1)
                        nc.vector.tensor_scalar(
                            out=s768[0:1, 0:1], in0=s768[0:1, 0:1],
                            scalar1=alpha, scalar2=None, op0=AL.mult,
                        )
                    mul_red(t2[:, 0:1], qim[:], kstage)    # sum QIm.KIm
                    mul_red(t3[:, 0:1], qre[:], kstage)    # sum QRe.KIm
                    # S_re col r = 2a*(tre + t2);  S_im col 6+r = 2a*(tim - t3)
                    nc.vector.tensor_tensor(t2[:, 0:1], tre[:, 0:1], t2[:, 0:1], AL.add)
                    nc.vector.tensor_scalar(
                        out=s2d[:, r:r + 1], in0=t2[:, 0:1],
                        scalar1=2.0 * alpha, scalar2=None, op0=AL.mult,
                    )
                    nc.vector.tensor_tensor(
                        t3[:, 0:1], tim[:, 0:1], t3[:, 0:1], AL.subtract
                    )
                    nc.vector.tensor_scalar(
                        out=s2d[:, 6 + r:7 + r], in0=t3[:, 0:1],
                        scalar1=2.0 * alpha, scalar2=None, op0=AL.mult,
                    )
                    if r == 0:
                        # fix DC: naive col0 row0 = 2a*(Sre0 + Sre768) -> a*Sre0
                        nc.vector.tensor_scalar(
                            out=s2d[0:1, 0:1], in0=s2d[0:1, 0:1],
                            scalar1=0.5, scalar2=s768[0:1, 0:1],
                            op0=AL.mult, op1=AL.subtract,
                        )
                        # Nyquist slot (junk Im f=0): S'[768] = a*Sre768
                        nc.vector.tensor_copy(s2d[0:1, 6:7], s768[0:1, 0:1])

                m_order = [0, 6, 1, 7, 2, 8, 3, 9, 4, 10, 5, 11]
                for m in m_order:
                    qf = dftpsum.tile([P, D], F32, tag="qf")
                    kf = dftpsum.tile([P, D], F32, tag="kf")
                    for li in range(NC):
                        wv = _mm(wbig[:, (m * NC + li) * P:(m * NC + li + 1) * P])
                        st, sp = (li == 0), (li == NC - 1)
                        for nh in range(2):
                            sl = slice(nh * 512, (nh + 1) * 512)
                            nc.tensor.matmul(
                                qf[:, sl], wv,
                                _mm(qbig[:, li * D + nh * 512:li * D + (nh + 1) * 512]),
                                start=st, stop=sp,
                            )
                            nc.tensor.matmul(
                                kf[:, sl], wv,
                                _mm(kbig[:, li * D + nh * 512:li * D + (nh + 1) * 512]),
                                start=st, stop=sp,
                            )
                    qf_t[m], kf_t[m] = qf, kf
                    if m >= 6:
                        emit_pair_products(m - 6)

                # ---- irfft of own spectrum: mv2d[p, lc] ----
                # S' (P, NC) -> DRAM, then read back flattened (c p order) and
                # broadcast across all partitions: sbc[p', 128*c + p] = s2d[p, c]
                nc.sync.dma_start(ds1[:].rearrange("c p -> p c"), s2d[:])
                nc.sync.dma_start(
                    sbc[:],
                    ds1[:].rearrange("c p -> (c p)").unsqueeze(0).to_broadcast(
                        (P, L)
                    ),
                )
                wb4 = wbig[:].rearrange("p (a b c) -> p a b c", a=NC, b=NC)
                sb3 = sbc[:].rearrange("p (a c) -> p a c", a=NC)
                jk3 = junk[:].rearrange("p (a c) -> p a c", a=NC)
                jk23 = jk2[:].rearrange("p (a c) -> p a c", a=8)
                for lc in range(NC):
                    # mv2d[:, lc] = sum_pf W[t, pf] * S'[pf]; one strided DVE
                    # mult over (128, 12, 128) + one ACT accumulate.
                    nc.vector.tensor_tensor(jk3, wb4[:, :, lc, :], sb3, AL.mult)
                    nc.scalar.activation(
                        out=junk[:].rearrange("p (a c) -> p a c", a=NC), in_=jk3,
                        func=mybir.ActivationFunctionType.Copy,
                        accum_out=mv2d[:, lc:lc + 1],
                    )

            # ---- allreduce own mean_value across cores ----
            nc.sync.dma_start(cc_in[:].rearrange("c p -> p c"), mv2d[:])
            if single_core:
                nc.sync.dma_start(cc_out[:], cc_in[:])
            else:
                nc.gpsimd.collective_compute(
                    "AllReduce",
                    AL.add,
                    replica_groups=[list(range(B))],
                    ins=[cc_in[:].opt()],
                    outs=[cc_out[:].opt()],
                )
            nc.sync.dma_start(
                bm[0:1, :], cc_out[:].rearrange("c p -> (c p)").unsqueeze(0)
            )

            # ---- top-7 indices from batch-summed mean_value ----
            top8 = misc.tile([1, 8], F32, tag="top8")
            idx8 = misc.tile([1, 8], mybir.dt.uint32, tag="idx8")
            idxf = misc.tile([1, 8], F32, tag="idxf")
            nc.vector.max(top8[:], bm[0:1, :])
            nc.vector.max_index(idx8[:], top8[:], bm[0:1, :])
            nc.vector.tensor_copy(idxf[:], idx8[:])

            # ---- per-core weights: softmax(own mv at idx[0..6]) ----
            # 128-partition one-hot gathers against mv2d, then a single ones-
            # matmul for the partition reduction.
            idxd = dram.tile([1, 8], F32)
            irep = misc.tile([P, 8], F32, tag="irep")
            nc.sync.dma_start(idxd[:], idxf[0:1, :])
            nc.sync.dma_start(irep[:], idxd[0:1, :].to_broadcast((P, 8)))
            iota2d = misc.tile([P, NC], F32, tag="iota2d")
            nc.gpsimd.iota(
                iota2d[:], pattern=[[P, NC]], base=0, channel_multiplier=1,
                allow_small_or_imprecise_dtypes=True,
            )  # iota2d[p, c] = p + 128*c = flat t index
            oh2d = misc.tile([P, NC], F32, tag="oh2d")
            rgat = misc.tile([P, 8], F32, tag="rgat")
            for i in range(TOPK):
                nc.vector.tensor_scalar(
                    out=oh2d[:], in0=iota2d[:], scalar1=irep[:, i:i + 1],
                    scalar2=None, op0=AL.is_equal,
                )
                nc.vector.tensor_tensor(oh2d[:], oh2d[:], mv2d[:], AL.mult)
                nc.vector.tensor_reduce(
                    out=rgat[:, i:i + 1], in_=oh2d[:],
                    axis=mybir.AxisListType.X, op=AL.add,
                )
            ones = misc.tile([P, 1], F32, tag="ones")
            nc.vector.memset(ones[:], 1.0)
            wraw = misc.tile([1, 8], F32, tag="wraw")
            with tc.tile_pool(name="midpsum", bufs=1, space="PSUM") as midpsum:
                wps = midpsum.tile([1, 8], F32, tag="wps")
                nc.tensor.matmul(
                    wps[0:1, 0:TOPK], ones[:], rgat[:, 0:TOPK],
                    start=True, stop=True,
                )
                nc.scalar.copy(wraw[0:1, 0:TOPK], wps[0:1, 0:TOPK])
            negmax = misc.tile([1, 1], F32, tag="negmax")
            nc.vector.tensor_reduce(
                out=negmax[0:1, 0:1], in_=wraw[0:1, 0:TOPK],
                axis=mybir.AxisListType.X, op=AL.max, negate=True,
            )
            ew = misc.tile([1, 8], F32, tag="ew")
            sumw = misc.tile([1, 1], F32, tag="sumw")
            nc.scalar.activation(
                out=ew[0:1, 0:TOPK], in_=wraw[0:1, 0:TOPK],
                func=mybir.ActivationFunctionType.Exp,
                bias=negmax[0:1, 0:1], scale=1.0,
                accum_out=sumw[0:1, 0:1],
            )
            rsum = misc.tile([1, 1], F32, tag="rsum")
            nc.vector.reciprocal(rsum[0:1, 0:1], sumw[0:1, 0:1])
            wvec = misc.tile([1, 8], F32, tag="wvec")
            nc.vector.tensor_scalar(
                out=wvec[0:1, 0:TOPK], in0=ew[0:1, 0:TOPK],
                scalar1=rsum[0:1, 0:1], scalar2=None, op0=AL.mult,
            )

            # ---- v_tab: rep of shift reps per (g, i):  (1, 12*7) ----
            vt = misc.tile([1, NC * TOPK], F32, tag="vt")
            for g in range(NC):
                nc.vector.tensor_copy(
                    vt[0:1, g * TOPK:(g + 1) * TOPK], idxf[0:1, 0:TOPK]
                )
            giof = misc.tile([1, NC * TOPK], F32, tag="giof")
            nc.gpsimd.iota(
                giof[0:1, :].rearrange("o (g i) -> o g i", g=NC),
                pattern=[[-P, NC], [0, TOPK]], base=0, channel_multiplier=0,
                allow_small_or_imprecise_dtypes=True,
            )
            nc.vector.tensor_tensor(vt[:], vt[:], giof[:], AL.add)
            cwrap = misc.tile([1, NC * TOPK], F32, tag="cwrap")
            nc.vector.tensor_scalar(
                out=cwrap[:], in0=vt[:], scalar1=-768.0, scalar2=1536.0,
                op0=AL.is_lt, op1=AL.mult,
            )
            nc.vector.tensor_tensor(vt[:], vt[:], cwrap[:], AL.add)
            nc.vector.tensor_scalar(
                out=cwrap[:], in0=vt[:], scalar1=768.0, scalar2=1536.0,
                op0=AL.is_ge, op1=AL.mult,
            )
            nc.vector.tensor_tensor(vt[:], vt[:], cwrap[:], AL.subtract)

            # replicate v_tab and weights to all partitions (via DRAM bounce)
            vrep = misc.tile([P, NC * TOPK], F32, tag="vrep")
            wrep = misc.tile([P, TOPK], F32, tag="wrep")
            vtd = dram.tile([1, NC * TOPK], F32)
            wvd = dram.tile([1, TOPK], F32)
            nc.sync.dma_start(vtd[:], vt[0:1, :])
            nc.sync.dma_start(wvd[:], wvec[0:1, 0:TOPK])
            nc.sync.dma_start(vrep[:], vtd[0:1, :].to_broadcast((P, NC * TOPK)))
            nc.sync.dma_start(wrep[:], wvd[0:1, :].to_broadcast((P, TOPK)))

            # ---- build the 12 circulant weight blocks ----
            af = misc.tile([P, P], F32, tag="af")
            nc.gpsimd.iota(
                af[:], pattern=[[-1, P]], base=0, channel_multiplier=1,
                allow_small_or_imprecise_dtypes=True,
            )  # A[p, j] = p - j
            tmpw = misc.tile([P, P], F32, tag="tmpw")
            wblk = [
                misc.tile([P, P], F32, tag=f"wblk{g}", name=f"wblk{g}")
                for g in range(NC)
            ]
            tmpw2 = misc.tile([P, P], F32, tag="tmpw2")
            for g in range(NC):
                eng = nc.vector if g % 3 != 2 else nc.gpsimd
                tw = tmpw if g % 3 != 2 else tmpw2
                for i in range(TOPK):
                    dst = _mm(wblk[g][:]) if i == 0 else tw[:]
                    eng.tensor_scalar(
                        out=dst, in0=af[:],
                        scalar1=vrep[:, g * TOPK + i:g * TOPK + i + 1],
                        scalar2=wrep[:, i:i + 1],
                        op0=AL.is_equal, op1=AL.mult,
                    )
                    if i > 0:
                        eng.tensor_tensor(
                            _mm(wblk[g][:]), wblk[g][:], tw[:], AL.add
                        )

            # ---- aggregation: out_T = sum_U Wblk[(U-T)%12]^T @ V_U ----
            with (
                tc.tile_pool(name="vpool", bufs=1) as vpool,
                tc.tile_pool(name="aggpsum", bufs=4, space="PSUM") as aggpsum,
            ):
                vbig = vpool.tile([P, NC * D], F32, tag="vbig")
                for li in range(NC):
                    nc.sync.dma_start(
                        _mm(vbig[:, li * D:(li + 1) * D]),
                        _mm(v_in[li * P:(li + 1) * P, :]),
                    )
                for T in range(NC):
                    po = aggpsum.tile([P, D], F32, tag="agg")
                    for U in range(NC):
                        g = (U - T) % NC
                        st, sp = (U == 0), (U == NC - 1)
                        for nh in range(2):
                            sl = slice(nh * 512, (nh + 1) * 512)
                            nc.tensor.matmul(
                                po[:, sl], _mm(wblk[g][:]),
                                _mm(vbig[:, U * D + nh * 512:U * D + (nh + 1) * 512]),
                                start=st, stop=sp,
                            )
                    ot = outp.tile([P, D], F32, tag="ot")
                    nc.scalar.copy(ot[:], po[:])
                    nc.sync.dma_start(out_dram[T * P:(T + 1) * P, :], ot[:])

    nc.compile()
    return nc


_prog_cache = None


def _get_program():
    global _prog_cache
    if _prog_cache is None:
        _prog_cache = build_program()
    return _prog_cache


def kernel(queries, keys, values, attn_mask=0):
    nc = _get_program()
    q = np.ascontiguousarray(np.asarray(queries, dtype=np.float32).reshape(B, L, D))
    k = np.ascontiguousarray(np.asarray(keys, dtype=np.float32).reshape(B, L, D))
    v = np.ascontiguousarray(np.asarray(values, dtype=np.float32).reshape(B, L, D))
    in_maps = [{"q": q[c], "k": k[c], "v": v[c]} for c in range(B)]
    res = bass_utils.run_bass_kernel_spmd(nc, in_maps, core_ids=list(range(B)))
    out = np.stack([res.results[c]["out"] for c in range(B)])
    return out.reshape(B, L, H, E)


if __name__ == "__main__":
    prog = build_program()
    print("program built ok;", len(prog.m.functions[0].allocations), "allocations")



# revision 2
# speedup vs baseline: 1.0906x; 1.0906x over previous
"""Trainium2 Bass kernel for nn_AutoCorrelation — v3.

v2 + performance restructuring:
  - Load phase: Q blocks 0-3, K0, Q4-7, K1, Q8-11, K2..K11 DMA order;
    PE transposes evacuated in 512-col groups (4 per PSUM bank); H[u]
    issued as 3 column-slice groups gated on Q arrival.
  - Keep-alive dummy matmuls keep the PE p-state ramped (full 2.4GHz)
    through the serial middle section so the aggregation runs at rate.
  - Single doubled-M DRAM write (broadcast source), collective input DMA'd
    straight from PSUM, 1/D folded into the ones reduction matrix.
  - Gather-reduce matmul in plain fp32 (fp32r forbids tiny outputs).
  - Aggregation U-loop starts at U=T so early T's consume weight blocks
    in build order.
"""

import numpy as np

import bass_rust
import concourse.bass as bass
import concourse.mybir as mybir
import concourse.tile as tile
from concourse import bacc
from concourse import bass_utils

B, L, H, E = 8, 1536, 16, 64
D = H * E            # 1024
P = 128
NC = L // P          # 12 t-blocks
NE = D // P          # 8 ch-chunks
TOPK = 7
F32 = mybir.dt.float32
F32R = mybir.dt.float32r
AL = mybir.AluOpType
AX = mybir.AxisListType.X

# keep-alive dummy matmul counts (tuned against TimelineSim)
ND_SKEW = 61    # last H matmul -> corr-reduce ones-matmul
ND_TOPK = 63    # corr-reduce -> gather-reduce matmul
ND_WBLK = 92    # gather-reduce -> first aggregation matmul


def _mm(ap):
    return ap.bitcast(F32R)


def build_program(single_core: bool = False) -> bass.Bass:
    nc = bacc.Bacc(
        "TRN2",
        target_bir_lowering=False,
        debug=False,
        num_devices=1 if single_core else B,
        name="autocorr4",
        dynamic_dma_scratch_size=512,
    )

    q_in = nc.dram_tensor("q", [L, D], F32, kind="ExternalInput")
    k_in = nc.dram_tensor("k", [L, D], F32, kind="ExternalInput")
    v_in = nc.dram_tensor("v", [L, D], F32, kind="ExternalInput")
    out_dram = nc.dram_tensor("out", [L, D], F32, kind="ExternalOutput")

    inv_d = 1.0 / D

    with tile.TileContext(nc) as tc:
        with (
            tc.tile_pool(name="misc", bufs=1) as misc,
            tc.tile_pool(name="dram", bufs=1, space="DRAM") as dram,
            tc.tile_pool(name="vpool", bufs=1) as vpool,
            tc.tile_pool(name="outp", bufs=3) as outp,
        ):
            # ---- persistent tiles ----
            af = misc.tile([P, P], F32, tag="af")            # af[p,j] = p - j
            idn = misc.tile([P, P], F32R, tag="idn")         # identity (f32r)
            ones2 = misc.tile([P, P], F32R, tag="ones2")     # all-ones
            onesd = misc.tile([P, P], F32R, tag="onesd")     # all 1/D
            M = misc.tile([P, L], F32, tag="M")              # blockrot sum
            cskew = misc.tile([P, L], F32, tag="cskew")      # row-skewed M
            bmB = misc.tile([P, L], F32, tag="bmB")          # batch sum bcast
            mv2d = misc.tile([P, NC], F32, tag="mv2d")       # own mv (p,c)
            iota2d = misc.tile([P, NC], F32, tag="iota2d")
            ag2 = misc.tile([P, NC * P], F32, tag="ag2")     # (p-j+128g) mod L
            wblk_all = misc.tile([P, NC * P], F32, tag="wblkall")
            vbig = vpool.tile([P, NC * D], F32, tag="vbig")

            md = dram.tile([P, 2 * L], F32)                  # doubled M
            cc_in = dram.tile([1, L], F32)

            # ---- tiny prep on Pool/DVE (overlaps loads) ----
            nc.gpsimd.iota(af[:], pattern=[[-1, P]], base=0, channel_multiplier=1,
                           allow_small_or_imprecise_dtypes=True)
            nc.vector.tensor_scalar(out=idn[:], in0=af[:], scalar1=0.0,
                                    scalar2=None, op0=AL.is_equal)
            nc.gpsimd.tensor_scalar(out=ones2[:], in0=af[:], scalar1=0.0,
                                    scalar2=1.0, op0=AL.mult, op1=AL.add)
            nc.gpsimd.tensor_scalar(out=onesd[:], in0=af[:], scalar1=0.0,
                                    scalar2=inv_d, op0=AL.mult, op1=AL.add)
            nc.gpsimd.iota(iota2d[:], pattern=[[P, NC]], base=0,
                           channel_multiplier=1,
                           allow_small_or_imprecise_dtypes=True)
            # ag2[p, (g, j)] = (p - j + 128g) mod 1536, prebuilt for the
            # weight-block compares
            nc.gpsimd.iota(ag2[:].rearrange("p (g j) -> p g j", g=NC),
                           pattern=[[P, NC], [-1, P]], base=0,
                           channel_multiplier=1,
                           allow_small_or_imprecise_dtypes=True)
            agneg = cskew[:, 0:NC * P]  # cskew unused until the skew read
            nc.gpsimd.tensor_scalar(out=_mm(agneg), in0=ag2[:], scalar1=0.0,
                                    scalar2=1536.0, op0=AL.is_lt, op1=AL.mult)
            nc.gpsimd.tensor_tensor(ag2[:], ag2[:], agneg, AL.add)

            # ================= corr phase =================
            with (
                tc.tile_pool(name="qkT", bufs=1) as qkT,
                tc.tile_pool(name="stage", bufs=6) as stage,
                tc.tile_pool(name="trps", bufs=2, space="PSUM") as trps,
                tc.tile_pool(name="hps", bufs=2, space="PSUM") as hps,
            ):
                # u-major layout: qT[:, u*D + e*128 + j] = Q^T[e-chunk][:, u-block]
                qT = qkT.tile([P, NC * D], F32, tag="qT")
                kT = qkT.tile([P, NC * D], F32, tag="kT")
                stg: dict = {}

                def load_block(src, u, key):
                    st = stage.tile([P, D], F32, tag="stg")
                    nc.sync.dma_start(_mm(st[:]), _mm(src[u * P:(u + 1) * P, :]))
                    stg[key] = st

                def transpose_block(dstT, u, key):
                    st = stg.pop(key)
                    for half in range(2):
                        tp = trps.tile([P, 512], F32, tag="tp")
                        for c in range(4):
                            e = half * 4 + c
                            nc.tensor.transpose(
                                _mm(tp[:, c * P:(c + 1) * P]),
                                _mm(st[:, e * P:(e + 1) * P]), idn[:],
                            )
                        # contiguous 512-wide evac
                        nc.scalar.copy(
                            _mm(dstT[:, u * D + half * 512:u * D + (half + 1) * 512]),
                            tp[:],
                        )

                qT4 = qT[:].rearrange("p (u e j) -> p u e j", u=NC, e=NE)

                def h_slice(u, hp, s):
                    sl = slice(s * 512, (s + 1) * 512)
                    for e in range(NE):
                        w = _mm(kT[:, u * D + e * P:u * D + (e + 1) * P])
                        nc.tensor.matmul(
                            hp[:, sl], w,
                            _mm(qT4[:, 4 * s:4 * s + 4, e, :]),
                            start=(e == 0), stop=(e == NE - 1),
                        )

                # DMA order: Q0-3, K0, Q4-7, K1, Q8-11, K2..K11 (+V after)
                for u in range(4):
                    load_block(q_in, u, ("q", u))
                load_block(k_in, 0, ("k", 0))
                for u in range(4, 8):
                    load_block(q_in, u, ("q", u))
                load_block(k_in, 1, ("k", 1))
                for u in range(8, 12):
                    load_block(q_in, u, ("q", u))
                for u in range(2, NC):
                    load_block(k_in, u, ("k", u))
                for li in range(NC):
                    nc.sync.dma_start(
                        _mm(vbig[:, li * D:(li + 1) * D]),
                        _mm(v_in[li * P:(li + 1) * P, :]),
                    )

                # PE order: trQ0-3, trK0, H0s0, trQ4-7, H0s1, trQ8-11, H0s2,
                #           trK1, H1, trK2, H2, ... trK11, H11
                for u in range(4):
                    transpose_block(qT, u, ("q", u))
                transpose_block(kT, 0, ("k", 0))
                hp0 = hps.tile([P, L], F32, tag="hp")
                h_slice(0, hp0, 0)
                for u in range(4, 8):
                    transpose_block(qT, u, ("q", u))
                h_slice(0, hp0, 1)
                transpose_block(kT, 1, ("k", 1))
                for u in range(8, 12):
                    transpose_block(qT, u, ("q", u))
                h_slice(0, hp0, 2)
                nc.vector.tensor_copy(M[:], hp0[:])
                def md_write(c0, c1):
                    # doubled write of M columns [c0, c1): two plain copies
                    nc.sync.dma_start(md[:, c0:c1], M[:, c0:c1])
                    nc.sync.dma_start(md[:, L + c0:L + c1], M[:, c0:c1])

                for u in range(1, NC - 1):
                    # transpose K[u+1] ahead so its evac hides under H[u]
                    if u + 1 < NC:
                        transpose_block(kT, u + 1, ("k", u + 1))
                    hp = hps.tile([P, L], F32, tag="hp")
                    for s in range(3):
                        h_slice(u, hp, s)
                    r = P * u
                    nc.vector.tensor_tensor(
                        M[:, 0:L - r], M[:, 0:L - r], hp[:, r:L], AL.add
                    )
                    nc.vector.tensor_tensor(
                        M[:, L - r:L], M[:, L - r:L], hp[:, 0:r], AL.add
                    )
                # u = 11: slice-pipelined rotate + doubled md write
                hp = hps.tile([P, L], F32, tag="hp")
                h_slice(11, hp, 0)
                nc.vector.tensor_tensor(
                    M[:, P:P + 512], M[:, P:P + 512], hp[:, 0:512], AL.add
                )
                md_write(P, P + 512)
                h_slice(11, hp, 1)
                nc.vector.tensor_tensor(
                    M[:, P + 512:P + 1024], M[:, P + 512:P + 1024],
                    hp[:, 512:1024], AL.add
                )
                md_write(P + 512, P + 1024)
                h_slice(11, hp, 2)
                nc.vector.tensor_tensor(
                    M[:, P + 1024:L], M[:, P + 1024:L], hp[:, 1024:L - P], AL.add
                )
                nc.vector.tensor_tensor(
                    M[:, 0:P], M[:, 0:P], hp[:, L - P:L], AL.add
                )
                md_write(P + 1024, L)
                md_write(0, P)
                # first keep-alive dummies reuse the free hp ring buffer
                hpd = hps.tile([P, L], F32, tag="hp")
                for _ in range(ND_SKEW):
                    nc.tensor.matmul(hpd[:, 0:P], ones2[:], idn[:],
                                     start=True, stop=True,
                                     skip_group_check=True)

            # ================= middle =================
            with (
                tc.tile_pool(name="cps", bufs=1, space="PSUM") as cps,
                tc.tile_pool(name="dps", bufs=1, space="PSUM") as dps,
            ):
                dummyps = dps.tile([P, P], F32, tag="dummyps")

                def dummies(n):
                    for _ in range(n):
                        nc.tensor.matmul(dummyps[:], ones2[:], idn[:],
                                         start=True, stop=True,
                                         skip_group_check=True)

                sksrc = md[:, :].copy()
                sksrc.ap = bass_rust.VecI64Pair([[2 * L + 1, P], [1, L]])
                nc.sync.dma_start(_mm(cskew[:]), _mm(sksrc))

                corrps = cps.tile([P, L], F32, tag="corrps")
                for s in range(3):
                    sl = slice(s * 512, (s + 1) * 512)
                    nc.tensor.matmul(corrps[:, sl], onesd[:], _mm(cskew[:, sl]),
                                     start=True, stop=True)

                dummies(ND_TOPK)

                # own mean_value row: ACT evac then DMA to DRAM
                corr1 = misc.tile([1, L], F32, tag="corr1")
                nc.scalar.copy(corr1[0:1, :], corrps[0:1, :])
                nc.sync.dma_start(cc_in[0:1, :], corr1[0:1, :])
                # own mean_value gathered BEFORE the in-place allreduce
                nc.sync.dma_start(
                    mv2d[:], cc_in[0:1, :].rearrange("o (c p) -> (o p) c", p=P)
                )
                # exp of own mean_value ahead of the top-k (softmax without
                # max-subtraction; values are O(10) so fp32 exp is safe)
                emv = misc.tile([P, NC], F32, tag="emv")
                nc.scalar.activation(
                    out=emv[:], in_=mv2d[:],
                    func=mybir.ActivationFunctionType.Exp, scale=1.0,
                )
                if not single_core:
                    nc.gpsimd.collective_compute(
                        "AllReduce", AL.add,
                        replica_groups=[list(range(B))],
                        ins=[cc_in[:].opt()],
                        outs=[cc_in[:].opt()],
                    )
                nc.sync.dma_start(bmB[:], cc_in[0:1, :].to_broadcast((P, L)))

                top8 = misc.tile([P, 8], F32, tag="top8")
                idx8 = misc.tile([P, 8], mybir.dt.uint32, tag="idx8")
                idxf = misc.tile([P, 8], F32, tag="idxf")
                nc.vector.max(top8[:], bmB[:])
                nc.vector.max_index(idx8[:], top8[:], bmB[:])
                nc.vector.tensor_copy(idxf[:], idx8[:])

                # ---- per-partition weights: softmax(own mv at idx[0..6]) ----
                oh2d = misc.tile([P, NC], F32, tag="oh2d")
                rgat = misc.tile([P, 8], F32, tag="rgat")
                for i in range(TOPK):
                    nc.vector.scalar_tensor_tensor(
                        out=oh2d[:], in0=iota2d[:], scalar=idxf[:, i:i + 1],
                        in1=emv[:], op0=AL.is_equal, op1=AL.mult,
                        accum_out=rgat[:, i:i + 1],
                    )
                wraw = misc.tile([P, 8], F32, tag="wraw")
                wps = cps.tile([P, 8], F32, tag="wps")
                nc.tensor.matmul(wps[:, 0:TOPK], ones2[:].bitcast(F32),
                                 rgat[:, 0:TOPK], start=True, stop=True)
                nc.scalar.copy(wraw[:, 0:TOPK], wps[:, 0:TOPK])
                sumw = misc.tile([P, 1], F32, tag="sumw")
                nc.vector.tensor_reduce(
                    out=sumw[:], in_=wraw[:, 0:TOPK], axis=AX, op=AL.add,
                )
                rsum = misc.tile([P, 1], F32, tag="rsum")
                nc.vector.reciprocal(rsum[:], sumw[:])
                wvec = misc.tile([P, 8], F32, tag="wvec")
                nc.vector.tensor_scalar(
                    out=wvec[:, 0:TOPK], in0=wraw[:, 0:TOPK], scalar1=rsum[:],
                    scalar2=None, op0=AL.mult,
                )

                # ---- weight blocks via two wide column-half passes ----
                dummies(ND_WBLK)
                # M is dead after the doubled-DRAM write: reuse as scratch
                HALF = NC * P // 2
                tmpv = M[:, 0:HALF]
                tmpg = M[:, HALF:NC * P]
                for i in range(TOPK):
                    for eng, tw, sl in (
                        (nc.vector, tmpv, slice(0, HALF)),
                        (nc.gpsimd, tmpg, slice(HALF, NC * P)),
                    ):
                        dst = _mm(wblk_all[:, sl]) if i == 0 else tw
                        eng.tensor_scalar(
                            out=dst, in0=ag2[:, sl],
                            scalar1=idxf[:, i:i + 1],
                            scalar2=wvec[:, i:i + 1],
                            op0=AL.is_equal, op1=AL.mult,
                        )
                        if i > 0:
                            eng.tensor_tensor(
                                _mm(wblk_all[:, sl]), wblk_all[:, sl],
                                tw, AL.add,
                            )

            # ================= aggregation =================
            with tc.tile_pool(name="aggps", bufs=4, space="PSUM") as aggps:
                for T in range(NC):
                    po = aggps.tile([P, D], F32, tag="agg")
                    ot = outp.tile([P, D], F32, tag="ot")
                    for nh in range(2):
                        sl = slice(nh * 512, (nh + 1) * 512)
                        for j in range(NC):
                            U = (T + j) % NC
                            nc.tensor.matmul(
                                po[:, sl], _mm(wblk_all[:, j * P:(j + 1) * P]),
                                _mm(vbig[:, U * D + nh * 512:U * D + (nh + 1) * 512]),
                                start=(j == 0), stop=(j == NC - 1),
                            )
                        nc.scalar.copy(ot[:, sl], po[:, sl])
                        nc.sync.dma_start(
                            out_dram[T * P:(T + 1) * P,
                                     nh * 512:(nh + 1) * 512],
                            ot[:, sl],
                        )

    nc.compile()
    return nc


_prog_cache = None


def _get_program():
    global _prog_cache
    if _prog_cache is None:
        _prog_cache = build_program()
    return _prog_cache


def kernel(queries, keys, values, attn_mask=0):
    nc = _get_program()
    q = np.ascontiguousarray(np.asarray(queries, dtype=np.float32).reshape(B, L, D))
    k = np.ascontiguousarray(np.asarray(keys, dtype=np.float32).reshape(B, L, D))
    v = np.ascontiguousarray(np.asarray(values, dtype=np.float32).reshape(B, L, D))
    in_maps = [{"q": q[c], "k": k[c], "v": v[c]} for c in range(B)]
    res = bass_utils.run_bass_kernel_spmd(nc, in_maps, core_ids=list(range(B)))
    out = np.stack([res.results[c]["out"] for c in range(B)])
    return out.reshape(B, L, H, E)


if __name__ == "__main__":
    prog = build_program(single_core=True)
    print("program built ok")
    from concourse.timeline_sim import TimelineSim
    t = TimelineSim(prog).simulate()
    print(f"TimelineSim: {int(t)} ns")
